# revision 73
# baseline (speedup 1.0000x reference)
"""M2MRF module as a single fused mixed-precision GEMM on 8 TRN2 NeuronCores.

The reference chains two 1x1 convs with no nonlinearity between them, so
    y2 = W2 @ (W1 @ cols + b1) + b2 = W_eff @ cols + const,
with W_eff = W2 @ W1 precomputed on host (fp64). The device runs the
single GEMM y = W_eff @ cols per batch shard:
    cols = unfold(x[b], k=4, s=4)            # [1024, 16384]
    y    = W_eff @ cols                      # [256, 16384]
    out[b] = fold(y, k=2, s=2)               # [64, 256, 256]

Sharding: 8 cores = 4 batches x 2 L-halves (LSH = 8192 columns/core).

Precision scheme (all products share scale S = 128, one PSUM group):
 - channels 0..383 (e3 part, 3 chunks): x3 = e3m4(2x), w3 = e3m4(64W);
   plain fp8 matmuls (1 cycle/row).
 - channels 384..767 (full e4 part, 3 chunks): xa = e4m3(2x),
   xb = e4m3(2x - xa), wh = e4m3(64W), wl = e4m3(64W - wh). DoubleRow
   fp8 matmuls (0.5 cycle/row) compute wh@xa (bulk, chunk pairs) plus
   the correction wl@xa + wh@xb (paired per chunk), dropping only the
   tiny wl@xb term.
 - channels 768..895 (r1): xa only on the x side, but W keeps both
   terms (wl_r1 rides in a DoubleRow pair with r2's bulk).
 - channels 896..1023 (r2): bare e4m3 xa, no corrections.
PSUM accumulates everything in fp32; result is written as bf16 of
128*y and rescaled on host. Measured rel err vs the fp32 reference is
1.82e-2 (gate: 2e-2), identical on host and device.
"""
import sys

sys.path.insert(0, "/opt/trn_rl_repo")

import numpy as np
import ml_dtypes

import concourse.bass as bass
import concourse.bacc as bacc
import concourse.mybir as mybir
import concourse.tile as tile
from concourse.bass_utils import run_bass_kernel_spmd

P = 128
NT = 512            # free-dim tile (one PSUM bank of fp32)
LSH = 8192          # L per core
NTILES = LSH // NT  # 16
K3C = 3             # e3m4 chunks (channels 0..383)
K4C = 4             # e4m3 chunk-pairs in xab (3 full + (r1,r2))
COUT = 256

_BF16 = ml_dtypes.bfloat16
_E3 = ml_dtypes.float8_e3m4
_E4 = ml_dtypes.float8_e4m3
_DR = mybir.MatmulPerfMode.DoubleRow


def _build_nc(ntiles=NTILES):
    nc = bacc.Bacc("TRN2", target_bir_lowering=False)
    x3_dram = nc.dram_tensor("x3", [K3C, P, LSH], mybir.dt.float8e3, kind="ExternalInput")
    # xab planes: (xa_f0,xb_f0)(xa_f1,xb_f1)(xa_f2,xb_f2)(xa_r1,xa_r2)
    xab_dram = nc.dram_tensor("xab", [K4C, 2, P, LSH], mybir.dt.float8e4, kind="ExternalInput")
    # weights are partition-major in DRAM: per-partition runs of 1-2KB keep
    # DMA descriptors above the 512B full-bandwidth threshold
    w3_dram = nc.dram_tensor("w3", [P, K3C, COUT], mybir.dt.float8e3, kind="ExternalInput")
    # wlh plane pairs: (wl0,wh0)(wl1,wh1)(wl2,wh2)(wl_r1,wh_r1)(wh_r2,0)
    wlh_dram = nc.dram_tensor("wlh", [P, 5, 2, COUT], mybir.dt.float8e4, kind="ExternalInput")
    y_dram = nc.dram_tensor("y", [2, P, LSH], mybir.dt.bfloat16, kind="ExternalOutput")

    with tile.TileContext(nc) as tc:
        with (
            tc.tile_pool(name="resident", bufs=1) as res,
            tc.tile_pool(name="outp", bufs=4) as outp,
            tc.tile_pool(name="ps", bufs=4, space="PSUM") as ps,
            tc.tile_pool(name="wps", bufs=1, space="PSUM") as wps,
        ):
            # PE p-state warmup: stream dummy matmuls on a zeroed tile while
            # the first DMAs are in flight, so real matmuls start at full
            # clock (the PE ramps over its first 3us of continuous activity).
            wu_sb = res.tile([P, 2, NT // 2], mybir.dt.float8e4, tag="wu")
            nc.vector.memset(wu_sb[:], 0)
            wu_pt = wps.tile([P, NT // 2], mybir.dt.float32, tag="wps")
            for _ in range(12):
                nc.tensor.matmul(
                    wu_pt[:],
                    wu_sb[:, :, 0:P],
                    wu_sb[:],
                    start=True,
                    stop=True,
                    perf_mode=_DR,
                )

            w3_sb = res.tile([P, K3C, COUT], mybir.dt.float8e3, tag="w3")
            wlh_sb = res.tile([P, 5, 2, COUT], mybir.dt.float8e4, tag="wlh")
            x3_sb = res.tile([P, K3C, LSH], mybir.dt.float8e3, tag="x3")
            xab_sb = res.tile([P, K4C, 2, LSH], mybir.dt.float8e4, tag="xab")
            # DMA order tracks first use: w3 + tile-0 x3 feed the opening e3
            # matmuls, then tile-0 xab + wlh for its DoubleRow tail, then the
            # remaining L-slices in consumption order.
            # L-tiles: uniform 512 except the last 512 split in two, which
            # shortens the end-of-kernel copy+DMA tail.
            tiles = [(t * NT, NT) for t in range(ntiles - 1)]
            last = (ntiles - 1) * NT
            tiles += [(last, 256), (last + 256, 256)]

            def load_x(off, sz):
                nsl = slice(off, off + sz)
                nc.sync.dma_start(
                    x3_sb[:, :, nsl],
                    x3_dram.ap()[:, :, nsl].rearrange("k p l -> p k l"),
                )
                nc.sync.dma_start(
                    xab_sb[:, :, :, nsl],
                    xab_dram.ap()[:, :, :, nsl].rearrange("k a p l -> p k a l"),
                )

            # Input DMA stays on uniform 512-col slices (512B+ descriptor
            # runs avoid the sub-512B bandwidth penalty); the compute tiling
            # below may be finer — region tracking handles containment.
            # Order tracks first use: w3 + tile-0 x3 chunks feed the opening
            # e3 matmuls; wlh (small) lands before the bulkier tile-0 xab.
            nsl0 = slice(0, NT)
            nc.sync.dma_start(
                x3_sb[:, :, nsl0], x3_dram.ap()[:, :, nsl0].rearrange("k p l -> p k l")
            )
            nc.sync.dma_start(w3_sb[:], w3_dram.ap())
            nc.sync.dma_start(wlh_sb[:], wlh_dram.ap())
            nc.sync.dma_start(
                xab_sb[:, :, :, nsl0],
                xab_dram.ap()[:, :, :, nsl0].rearrange("k a p l -> p k a l"),
            )
            for nt in range(1, ntiles):
                load_x(nt * NT, NT)

            for off, sz in tiles:
                nsl = slice(off, off + sz)
                o_sb = outp.tile([P, 2, NT], mybir.dt.bfloat16, tag="o")
                for m in range(2):
                    msl = slice(m * P, (m + 1) * P)
                    pt = ps.tile([P, NT], mybir.dt.float32, tag="ps")
                    # e3 part: plain fp8 matmuls over 3 chunks
                    for k in range(K3C):
                        nc.tensor.matmul(
                            pt[:, :sz],
                            w3_sb[:, k, msl],
                            x3_sb[:, k, nsl],
                            start=(k == 0),
                            stop=False,
                        )
                    # e4 bulk pairs: (wh0@xa0 + wh1@xa1), (wh2@xa2 + wh_r1@xa_r1)
                    for q in range(2):
                        nc.tensor.matmul(
                            pt[:, :sz],
                            wlh_sb[:, 2 * q:2 * q + 2, 1, msl],
                            xab_sb[:, 2 * q:2 * q + 2, 0, nsl],
                            start=False,
                            stop=False,
                            perf_mode=_DR,
                        )
                    # r-pair: wl_r1 @ xa_r1 + wh_r2 @ xa_r2
                    nc.tensor.matmul(
                        pt[:, :sz],
                        wlh_sb[:, 3:5, 0, msl],
                        xab_sb[:, 3, :, nsl],
                        start=False,
                        stop=False,
                        perf_mode=_DR,
                    )
                    # full-chunk corrections: wl@xa + wh@xb, paired per chunk
                    for i in range(3):
                        nc.tensor.matmul(
                            pt[:, :sz],
                            wlh_sb[:, i, :, msl],
                            xab_sb[:, i, :, nsl],
                            start=False,
                            stop=(i == 2),
                            perf_mode=_DR,
                        )
                    nc.vector.tensor_copy(out=o_sb[:, m, :sz], in_=pt[:, :sz])
                # merged per-tile output DMA keeps instruction count low
                nc.scalar.dma_start(
                    y_dram.ap()[:, :, nsl].rearrange("m p l -> p m l"),
                    o_sb[:, :, :sz],
                )

    nc.finalize()
    return nc


_NC_CACHE = None


def kernel(x, W1, b1, W2, b2):
    global _NC_CACHE
    x = np.asarray(x)
    W1, b1 = np.asarray(W1), np.asarray(b1)
    W2, b2 = np.asarray(W2), np.asarray(b2)
    n, c, h, w = x.shape  # 4, 64, 512, 512

    # ---- host unfold: cols[b, c*16+kh*4+kw, ph*128+pw] = x[b,c,ph*4+kh,pw*4+kw]
    cols = x.reshape(n, c, 128, 4, 128, 4).transpose(0, 1, 3, 5, 2, 4)
    cols = np.ascontiguousarray(cols).reshape(n, 1024, 16384)

    # ---- fold the two GEMMs into one; quantize weights (shared scale 64)
    Weff = (W2.astype(np.float64) @ W1.astype(np.float64)).astype(np.float32)
    Wt64 = np.ascontiguousarray(Weff.T) * np.float32(64.0)  # [1024, 256]
    w3 = np.ascontiguousarray(
        Wt64[:384].astype(_E3).reshape(K3C, P, COUT).transpose(1, 0, 2)
    )  # [P, K3C, COUT]
    wh = Wt64[384:].astype(_E4)                        # 5 chunks: f0..f2,r1,r2
    wl = (Wt64[384:] - wh.astype(np.float32)).astype(_E4)
    wh5 = wh.reshape(5, P, COUT)
    wl5 = wl.reshape(5, P, COUT)
    # plane pairs: (wl_f0,wh_f0)(wl_f1,wh_f1)(wl_f2,wh_f2)(wl_r1,wh_r1)(wh_r2,0)
    wlh = np.zeros((5, 2, P, COUT), dtype=_E4)
    for kk in range(4):
        wlh[kk, 0] = wl5[kk]
        wlh[kk, 1] = wh5[kk]
    wlh[4, 0] = wh5[4]
    wlh = np.ascontiguousarray(wlh.transpose(2, 0, 1, 3))  # [P, 5, 2, COUT]

    if _NC_CACHE is None:
        _NC_CACHE = _build_nc()
    nc = _NC_CACHE

    in_maps = []
    for core in range(8):
        b, half = core // 2, core % 2
        cs = cols[b, :, half * LSH:(half + 1) * LSH] * np.float32(2.0)
        x3 = np.ascontiguousarray(cs[:384]).astype(_E3).reshape(K3C, P, LSH)
        x4 = np.ascontiguousarray(cs[384:])               # 5 chunks
        xa = x4.astype(_E4)
        xb = (x4[:384] - xa[:384].astype(np.float32)).astype(_E4)
        # planes: (xa_f0,xb_f0)(xa_f1,xb_f1)(xa_f2,xb_f2)(xa_r1,xa_r2)
        xa5 = xa.reshape(5, P, LSH)
        xab = np.empty((K4C, 2, P, LSH), dtype=_E4)
        xb3 = xb.reshape(3, P, LSH)
        for kk in range(3):
            xab[kk, 0] = xa5[kk]
            xab[kk, 1] = xb3[kk]
        xab[3, 0] = xa5[3]
        xab[3, 1] = xa5[4]
        xab = np.ascontiguousarray(xab)
        in_maps.append({"x3": x3, "xab": xab, "w3": w3, "wlh": wlh})

    res = run_bass_kernel_spmd(nc, in_maps, core_ids=list(range(8)))

    # ---- gather + rescale (device computed 128*y) + fold on host
    y2 = np.empty((n, COUT, 16384), dtype=np.float32)
    for core in range(8):
        b, half = core // 2, core % 2
        y2[b, :, half * LSH:(half + 1) * LSH] = (
            res.results[core]["y"].reshape(COUT, LSH).astype(np.float32)
        )
    y2 *= np.float32(1.0 / 128.0)

    # bias epilogue (b1/b2 are zeros in this problem; exact otherwise)
    v = W2.astype(np.float64) @ b1.astype(np.float64) + b2.astype(np.float64)
    if np.any(v):
        y2 += v.astype(np.float32)[None, :, None]

    out = y2.reshape(n, c, 2, 2, 128, 128).transpose(0, 1, 4, 2, 5, 3)
    return np.ascontiguousarray(out).reshape(n, c, 256, 256)
